# revision 1
# baseline (speedup 1.0000x reference)
"""Logsparse attention Trainium2 kernel.

Problem: B=4 H=8 L=4096 E=64, mask = causal & (dist <= win_len | dist is pow2).

Structure exploited: with 128-row query blocks b and 128-row key blocks,
query block b only interacts with key blocks {b, b-1, b-2, b-4, b-8, b-16}:
  - blocks b, b-1 carry the sliding window (win_len <= 127) plus pow2 dists
    {1..128} (dense-ish mask),
  - blocks b-2, b-4, b-8, b-16 carry exactly the pow2 dists 256/512/1024/2048,
    whose in-block mask is the pure diagonal kk == qq.

Sharding: B*H = 32 heads, 4 per core (8 cores). Heads processed in pairs:
the pair's [L, 2*64] q/k matrices are DMA-xbar-transposed into [128, L]
(e-on-partition) so QK^T matmuls contract over e; the two heads occupy
partition halves and run as row-packed (tile_position) concurrent matmuls.

Softmax: no max-subtraction (scores ~N(0,1), exp is safe in fp32/bf16);
denominator comes for free from a ones-column appended to V.
Compute dtype bf16 (inputs cast on host), accumulation fp32 (PSUM).
"""

import os
import sys
from contextlib import ExitStack

import numpy as np

for _p in ("/opt/trn_rl_repo", "/root/.axon_site/_ro/trn_rl_repo"):
    if os.path.isdir(_p) and _p not in sys.path:
        sys.path.insert(0, _p)

import ml_dtypes  # noqa: E402
import concourse.bass as bass  # noqa: E402
import concourse.tile as tile  # noqa: E402
from concourse import bacc, mybir  # noqa: E402
from concourse.bass import ds  # noqa: E402
from concourse.bass_utils import run_bass_kernel_spmd  # noqa: E402

B, H, L, E = 4, 8, 4096, 64
NCORES = 8
BH = B * H                  # 32 heads total
BH_PER_CORE = BH // NCORES  # 4
NPAIRS = BH_PER_CORE // 2   # 2 head-pairs per core
NB = L // 128               # 32 query/key blocks
DELTAS = (0, 1, 2, 4, 8, 16)
NSLOT = len(DELTAS)
OUT_NB = 4                  # query blocks batched per output DMA
SCALE = 1.0 / float(np.sqrt(E))
BF16 = ml_dtypes.bfloat16

_NC_CACHE = {}


def _active(b):
    return [(j, d) for j, d in enumerate(DELTAS) if b - d >= 0]


def _kernel_body(ctx, tc, q2, k2, v, maskt, out):
    nc = tc.nc
    consts = ctx.enter_context(tc.tile_pool(name="consts", bufs=1))
    pairbuf = ctx.enter_context(tc.tile_pool(name="pair", bufs=2))
    ppool = ctx.enter_context(tc.tile_pool(name="pexp", bufs=5))
    pmpool = ctx.enter_context(tc.tile_pool(name="pmask", bufs=5))
    spool = ctx.enter_context(tc.tile_pool(name="spsum", bufs=2, space="PSUM"))
    opool = ctx.enter_context(tc.tile_pool(name="opsum", bufs=2, space="PSUM"))
    rpool = ctx.enter_context(tc.tile_pool(name="rtile", bufs=4))
    outpool = ctx.enter_context(tc.tile_pool(name="outsb", bufs=3))

    mask_sb = consts.tile([128, 2, NSLOT, 128], mybir.dt.bfloat16)
    nc.sync.dma_start(out=mask_sb[:], in_=maskt[:])

    # Hoist all input loads, split across the two HWDGE queues. Loads are
    # chunked so the first score matmuls unblock after a small fraction of
    # the traffic (Tile tracks per-region accesses).
    qTs, kTs, vexts = [], [], []
    CH = L // 4
    for pr in range(NPAIRS):
        # host pre-transposed [128, L]: partition = (head, e)
        qT = pairbuf.tile([128, L], mybir.dt.bfloat16, tag=f"qT{pr}")
        kT = pairbuf.tile([128, L], mybir.dt.bfloat16, tag=f"kT{pr}")
        # V with a ones column appended per block: [128, h, block, 65]
        vext = pairbuf.tile([128, 2, NB, 65], mybir.dt.bfloat16, tag=f"vext{pr}")
        nc.vector.memset(vext[:, :, :, 64], 1.0)
        nbc = NB // 4
        for c in range(4):
            nc.scalar.dma_start(
                out=qT[:, ds(c * CH, CH)], in_=q2[pr][:, ds(c * CH, CH)]
            )
            nc.sync.dma_start(
                out=kT[:, ds(c * CH, CH)], in_=k2[pr][:, ds(c * CH, CH)]
            )
            for h in range(2):
                nc.sync.dma_start(
                    out=vext[:, h, ds(c * nbc, nbc), 0:64],
                    in_=v[2 * pr + h][ds(c * nbc * 128, nbc * 128), :].rearrange(
                        "(n p) e -> p n e", p=128
                    ),
                )
        qTs.append(qT)
        kTs.append(kT)
        vexts.append(vext)

    # Software-pipelined emission (PE queue is in-order): scores of item t
    # are issued before exp/mask of t-1 and PV of t-2, so the PE always has
    # independent matmul work while ACT/DVE process earlier blocks.
    items = [(pr, b) for pr in range(NPAIRS) for b in range(NB)]
    st = {}
    out_sbs = {}

    def emit_scores(pr, b):
        S = spool.tile([128, 2, NSLOT, 128], mybir.dt.float32, tag="S")
        kT, qT = kTs[pr], qTs[pr]
        for j, d in _active(b):
            for h in range(2):
                nc.tensor.matmul(
                    S[:, h, j, :],
                    lhsT=kT[64 * h : 64 * h + 64, ds(128 * (b - d), 128)],
                    rhs=qT[64 * h : 64 * h + 64, ds(128 * b, 128)],
                    start=True,
                    stop=True,
                    tile_position=(64 * h, 0),
                )
        st[(pr, b)] = S

    def emit_expmask(pr, b):
        S = st.pop((pr, b))
        P = ppool.tile([128, 2, NSLOT, 128], mybir.dt.bfloat16, tag="P")
        nc.scalar.activation(
            P[:], S[:], mybir.ActivationFunctionType.Exp, scale=SCALE
        )
        PM = pmpool.tile([128, 2, NSLOT, 128], mybir.dt.bfloat16, tag="PM")
        nc.vector.tensor_mul(PM[:], P[:], mask_sb[:])
        st[(pr, b, "PM")] = PM

    def emit_pv(pr, b):
        PM = st.pop((pr, b, "PM"))
        vext = vexts[pr]
        acts = _active(b)
        O = opool.tile([128, 2, 65], mybir.dt.float32, tag="O")
        for h in range(2):
            for i, (j, d) in enumerate(acts):
                nc.tensor.matmul(
                    O[:, h, :],
                    lhsT=PM[:, h, j, :],
                    rhs=vext[:, h, b - d, :],
                    start=(i == 0),
                    stop=(i == len(acts) - 1),
                )
        r = rpool.tile([128, 2], mybir.dt.float32, tag="r")
        nc.vector.reciprocal(r[:], O[:, :, 64])
        if b % OUT_NB == 0:
            out_sbs[pr] = outpool.tile(
                [128, OUT_NB, 2, 64], mybir.dt.float32, tag="osb", name="osb"
            )
        out_sb = out_sbs[pr]
        nc.vector.tensor_mul(
            out_sb[:, b % OUT_NB, :, :],
            O[:, :, 0:64],
            r[:].to_broadcast([128, 2, 64]),
        )
        if b % OUT_NB == OUT_NB - 1:
            w0 = b - (OUT_NB - 1)
            for h in range(2):
                nc.sync.dma_start(
                    out=out[2 * pr + h][ds(128 * w0, OUT_NB * 128), :].rearrange(
                        "(n p) e -> p n e", p=128
                    ),
                    in_=out_sb[:, :, h, :],
                )

    for t, (pr, b) in enumerate(items):
        emit_scores(pr, b)
        if t >= 1:
            emit_expmask(*items[t - 1])
        if t >= 2:
            emit_pv(*items[t - 2])
    emit_expmask(*items[-1])
    emit_pv(*items[-2])
    emit_pv(*items[-1])


def _build_nc():
    key = "v1"
    if key in _NC_CACHE:
        return _NC_CACHE[key]
    nc = bacc.Bacc(
        "TRN2",
        target_bir_lowering=False,
        debug=False,
        enable_asserts=False,
        num_devices=NCORES,
    )
    q2 = nc.dram_tensor("q2", [NPAIRS, 128, L], mybir.dt.bfloat16, kind="ExternalInput")
    k2 = nc.dram_tensor("k2", [NPAIRS, 128, L], mybir.dt.bfloat16, kind="ExternalInput")
    v = nc.dram_tensor(
        "v", [BH_PER_CORE, L, E], mybir.dt.bfloat16, kind="ExternalInput"
    )
    maskt = nc.dram_tensor(
        "maskt", [128, 2, NSLOT, 128], mybir.dt.bfloat16, kind="ExternalInput"
    )
    out = nc.dram_tensor(
        "out", [BH_PER_CORE, L, E], mybir.dt.float32, kind="ExternalOutput"
    )
    with tile.TileContext(nc) as tc, ExitStack() as ctx:
        _kernel_body(ctx, tc, q2.ap(), k2.ap(), v.ap(), maskt.ap(), out.ap())
    nc.compile()
    _NC_CACHE[key] = nc
    return nc


def _mask_tiles(win):
    kk = np.arange(128, dtype=np.int64)[:, None]
    qq = np.arange(128, dtype=np.int64)[None, :]
    tiles = np.zeros((128, 2, NSLOT, 128), np.float32)
    for j, d in enumerate(DELTAS):
        dist = 128 * d + qq - kk
        pow2 = (dist > 0) & ((dist & (dist - 1)) == 0)
        ok = (dist >= 0) & ((dist <= win) | pow2)
        tiles[:, 0, j, :] = ok
        tiles[:, 1, j, :] = ok
    return tiles.astype(BF16)


def _run(q, k, v, win_len, trace=False):
    win = int(np.asarray(win_len))
    assert 0 <= win < 128, f"win_len {win} out of supported range [0, 128)"
    q = np.asarray(q, dtype=np.float32).reshape(BH, L, E)
    k = np.asarray(k, dtype=np.float32).reshape(BH, L, E)
    v = np.asarray(v, dtype=np.float32).reshape(BH, L, E)
    maskt = _mask_tiles(win)

    in_maps = []
    for c in range(NCORES):
        sl = slice(BH_PER_CORE * c, BH_PER_CORE * (c + 1))
        qc = q[sl].astype(BF16)  # [4, L, E]
        kc = k[sl].astype(BF16)
        vc = v[sl].astype(BF16)
        # pack head pairs on partitions, pre-transposed: [pairs, (h e), L]
        q2 = np.ascontiguousarray(
            qc.reshape(NPAIRS, 2, L, E).transpose(0, 1, 3, 2).reshape(NPAIRS, 128, L)
        )
        k2 = np.ascontiguousarray(
            kc.reshape(NPAIRS, 2, L, E).transpose(0, 1, 3, 2).reshape(NPAIRS, 128, L)
        )
        in_maps.append({"q2": q2, "k2": k2, "v": vc, "maskt": maskt})

    nc = _build_nc()
    res = run_bass_kernel_spmd(nc, in_maps, core_ids=list(range(NCORES)), trace=trace)
    outs = np.stack([res.results[c]["out"] for c in range(NCORES)])  # [8,4,L,E]
    full = outs.reshape(B, H, L, E).astype(np.float32)
    return full, res


def kernel(q, k, v, win_len):
    out, _ = _run(q, k, v, win_len, trace=False)
    return out



# revision 2
# speedup vs baseline: 1.0381x; 1.0381x over previous
"""Logsparse attention Trainium2 kernel.

Problem: B=4 H=8 L=4096 E=64, mask = causal & (dist <= win_len | dist is pow2).

Structure exploited: with 128-row query blocks b and 128-row key blocks,
query block b only interacts with key blocks {b, b-1, b-2, b-4, b-8, b-16}:
  - blocks b, b-1 carry the sliding window (win_len <= 127) plus pow2 dists
    {1..128} (dense-ish mask),
  - blocks b-2, b-4, b-8, b-16 carry exactly the pow2 dists 256/512/1024/2048,
    whose in-block mask is the pure diagonal kk == qq.

Sharding: B*H = 32 heads, 4 per core (8 cores). Heads processed in pairs:
the pair's [L, 2*64] q/k matrices are host-transposed into [128, L]
(e-on-partition) so QK^T matmuls contract over e; the two heads occupy
partition halves and run as row-packed (tile_position) concurrent matmuls.

Softmax: no max-subtraction (scores ~N(0,1), exp is safe in fp32/bf16);
denominator comes for free from a ones-column appended to V.
Compute dtype bf16 (inputs cast on host), accumulation fp32 (PSUM).

All HBM traffic is partition-major and contiguous per partition (v gets its
ones column and [128, 2, NB, 65] layout on the host; the output is stored
as [128, NB, 2, OUT_NB, 64] bf16 and un-permuted on the host) so DMAs don't
fragment into tiny descriptors. Input DMas are emitted need-first so the
first score matmul unblocks after ~3 small transfers.
"""

import os
import sys
from contextlib import ExitStack

import numpy as np

for _p in ("/opt/trn_rl_repo", "/root/.axon_site/_ro/trn_rl_repo"):
    if os.path.isdir(_p) and _p not in sys.path:
        sys.path.insert(0, _p)

import ml_dtypes  # noqa: E402
import concourse.bass as bass  # noqa: E402
import concourse.tile as tile  # noqa: E402
from concourse import bacc, mybir  # noqa: E402
from concourse.bass import ds  # noqa: E402
from concourse.bass_utils import run_bass_kernel_spmd  # noqa: E402

B, H, L, E = 4, 8, 4096, 64
NCORES = 8
BH = B * H                  # 32 heads total
BH_PER_CORE = BH // NCORES  # 4
NPAIRS = BH_PER_CORE // 2   # 2 head-pairs per core
NB = L // 128               # 32 query/key blocks
DELTAS = (0, 1, 2, 4, 8, 16)
NSLOT = len(DELTAS)
OUT_NB = 4                  # query blocks batched per output DMA
SCALE = 1.0 / float(np.sqrt(E))
BF16 = ml_dtypes.bfloat16

_NC_CACHE = {}


def _active(b):
    return [(j, d) for j, d in enumerate(DELTAS) if b - d >= 0]


def _kernel_body(ctx, tc, q2, k2, vext_in, maskt, out):
    nc = tc.nc
    consts = ctx.enter_context(tc.tile_pool(name="consts", bufs=1))
    pairbuf = ctx.enter_context(tc.tile_pool(name="pair", bufs=2))
    ppool = ctx.enter_context(tc.tile_pool(name="pexp", bufs=5))
    pmpool = ctx.enter_context(tc.tile_pool(name="pmask", bufs=5))
    spool = ctx.enter_context(tc.tile_pool(name="spsum", bufs=2, space="PSUM"))
    opool = ctx.enter_context(tc.tile_pool(name="opsum", bufs=2, space="PSUM"))
    rpool = ctx.enter_context(tc.tile_pool(name="rtile", bufs=4))
    outpool = ctx.enter_context(tc.tile_pool(name="outsb", bufs=3))

    mask_sb = consts.tile([128, 2, NSLOT, 128], mybir.dt.bfloat16)
    nc.sync.dma_start(out=mask_sb[:], in_=maskt[:])

    # Hoist all input loads, split across the two HWDGE queues, emitted in
    # need-order: the first chunks are small so item 0's matmuls unblock
    # after ~3 small transfers (Tile tracks per-region accesses).
    qTs, kTs, vexts = [], [], []
    for pr in range(NPAIRS):
        qT = pairbuf.tile([128, L], mybir.dt.bfloat16, tag=f"qT{pr}")
        kT = pairbuf.tile([128, L], mybir.dt.bfloat16, tag=f"kT{pr}")
        vext = pairbuf.tile([128, 2, NB, 65], mybir.dt.bfloat16, tag=f"vext{pr}")
        qTs.append(qT)
        kTs.append(kT)
        vexts.append(vext)

    # chunk boundaries in key/query blocks: small head chunks first
    CHUNKS = ((0, 2), (2, 6), (6, 14), (14, 23), (23, 32))
    for lo, hi in CHUNKS:
        for pr in range(NPAIRS):
            nc.scalar.dma_start(
                out=qTs[pr][:, ds(lo * 128, (hi - lo) * 128)],
                in_=q2[pr][:, ds(lo * 128, (hi - lo) * 128)],
            )
            nc.sync.dma_start(
                out=kTs[pr][:, ds(lo * 128, (hi - lo) * 128)],
                in_=k2[pr][:, ds(lo * 128, (hi - lo) * 128)],
            )
            nc.sync.dma_start(
                out=vexts[pr][:, :, ds(lo, hi - lo), :],
                in_=vext_in[pr][:, :, ds(lo, hi - lo), :],
            )

    # Software-pipelined emission (PE queue is in-order): scores of item t
    # are issued before exp/mask of t-1 and PV of t-2, so the PE always has
    # independent matmul work while ACT/DVE process earlier blocks.
    items = [(pr, b) for pr in range(NPAIRS) for b in range(NB)]
    st = {}
    out_sbs = {}

    def emit_scores(pr, b):
        S = spool.tile([128, 2, NSLOT, 128], mybir.dt.float32, tag="S")
        kT, qT = kTs[pr], qTs[pr]
        for j, d in _active(b):
            for h in range(2):
                nc.tensor.matmul(
                    S[:, h, j, :],
                    lhsT=kT[64 * h : 64 * h + 64, ds(128 * (b - d), 128)],
                    rhs=qT[64 * h : 64 * h + 64, ds(128 * b, 128)],
                    start=True,
                    stop=True,
                    tile_position=(64 * h, 0),
                )
        st[(pr, b)] = S

    def emit_expmask(pr, b):
        S = st.pop((pr, b))
        P = ppool.tile([128, 2, NSLOT, 128], mybir.dt.bfloat16, tag="P")
        nc.scalar.activation(
            P[:], S[:], mybir.ActivationFunctionType.Exp, scale=SCALE
        )
        PM = pmpool.tile([128, 2, NSLOT, 128], mybir.dt.bfloat16, tag="PM")
        nc.vector.tensor_mul(PM[:], P[:], mask_sb[:])
        st[(pr, b, "PM")] = PM

    def emit_pv(pr, b):
        PM = st.pop((pr, b, "PM"))
        vext = vexts[pr]
        acts = _active(b)
        O = opool.tile([128, 2, 65], mybir.dt.float32, tag="O")
        for h in range(2):
            for i, (j, d) in enumerate(acts):
                nc.tensor.matmul(
                    O[:, h, :],
                    lhsT=PM[:, h, j, :],
                    rhs=vext[:, h, b - d, :],
                    start=(i == 0),
                    stop=(i == len(acts) - 1),
                )
        r = rpool.tile([128, 2], mybir.dt.float32, tag="r")
        nc.vector.reciprocal(r[:], O[:, :, 64])
        if b % OUT_NB == 0:
            out_sbs[pr] = outpool.tile(
                [128, OUT_NB, 2, 64], mybir.dt.bfloat16, tag="osb", name="osb"
            )
        out_sb = out_sbs[pr]
        nc.vector.tensor_mul(
            out_sb[:, b % OUT_NB, :, :],
            O[:, :, 0:64],
            r[:].to_broadcast([128, 2, 64]),
        )
        if b % OUT_NB == OUT_NB - 1:
            w0 = b - (OUT_NB - 1)
            nc.sync.dma_start(
                out=out[pr][:, ds(w0, OUT_NB), :, :],
                in_=out_sb[:],
            )

    for t, (pr, b) in enumerate(items):
        emit_scores(pr, b)
        if t >= 1:
            emit_expmask(*items[t - 1])
        if t >= 2:
            emit_pv(*items[t - 2])
    emit_expmask(*items[-1])
    emit_pv(*items[-2])
    emit_pv(*items[-1])


def _build_nc():
    key = "v2"
    if key in _NC_CACHE:
        return _NC_CACHE[key]
    nc = bacc.Bacc(
        "TRN2",
        target_bir_lowering=False,
        debug=False,
        enable_asserts=False,
        num_devices=NCORES,
    )
    q2 = nc.dram_tensor("q2", [NPAIRS, 128, L], mybir.dt.bfloat16, kind="ExternalInput")
    k2 = nc.dram_tensor("k2", [NPAIRS, 128, L], mybir.dt.bfloat16, kind="ExternalInput")
    vext_in = nc.dram_tensor(
        "vext", [NPAIRS, 128, 2, NB, 65], mybir.dt.bfloat16, kind="ExternalInput"
    )
    maskt = nc.dram_tensor(
        "maskt", [128, 2, NSLOT, 128], mybir.dt.bfloat16, kind="ExternalInput"
    )
    out = nc.dram_tensor(
        "out", [NPAIRS, 128, NB, 2, 64], mybir.dt.bfloat16, kind="ExternalOutput"
    )
    with tile.TileContext(nc) as tc, ExitStack() as ctx:
        _kernel_body(ctx, tc, q2.ap(), k2.ap(), vext_in.ap(), maskt.ap(), out.ap())
    nc.compile()
    _NC_CACHE[key] = nc
    return nc


def _mask_tiles(win):
    kk = np.arange(128, dtype=np.int64)[:, None]
    qq = np.arange(128, dtype=np.int64)[None, :]
    tiles = np.zeros((128, 2, NSLOT, 128), np.float32)
    for j, d in enumerate(DELTAS):
        dist = 128 * d + qq - kk
        pow2 = (dist > 0) & ((dist & (dist - 1)) == 0)
        ok = (dist >= 0) & ((dist <= win) | pow2)
        tiles[:, 0, j, :] = ok
        tiles[:, 1, j, :] = ok
    return tiles.astype(BF16)


def _run(q, k, v, win_len, trace=False):
    win = int(np.asarray(win_len))
    assert 0 <= win < 128, f"win_len {win} out of supported range [0, 128)"
    q = np.asarray(q, dtype=np.float32).reshape(BH, L, E)
    k = np.asarray(k, dtype=np.float32).reshape(BH, L, E)
    v = np.asarray(v, dtype=np.float32).reshape(BH, L, E)
    maskt = _mask_tiles(win)

    in_maps = []
    for c in range(NCORES):
        sl = slice(BH_PER_CORE * c, BH_PER_CORE * (c + 1))
        qc = q[sl].astype(BF16)  # [4, L, E]
        kc = k[sl].astype(BF16)
        vc = v[sl].astype(BF16)
        # pack head pairs on partitions, pre-transposed: [pairs, (h e), L]
        q2 = np.ascontiguousarray(
            qc.reshape(NPAIRS, 2, L, E).transpose(0, 1, 3, 2).reshape(NPAIRS, 128, L)
        )
        k2 = np.ascontiguousarray(
            kc.reshape(NPAIRS, 2, L, E).transpose(0, 1, 3, 2).reshape(NPAIRS, 128, L)
        )
        # v packed partition-major with the ones column baked in:
        # [pr, 128, 2, NB, 65];  v row 128*n+p of head (pr,h) -> [pr, p, h, n, 0:64]
        vx = np.ones((NPAIRS, 2, NB, 128, 65), np.float32).astype(BF16)
        vx[:, :, :, :, 0:64] = vc.reshape(NPAIRS, 2, NB, 128, E)
        vext = np.ascontiguousarray(vx.transpose(0, 3, 1, 2, 4))
        in_maps.append({"q2": q2, "k2": k2, "vext": vext, "maskt": maskt})

    nc = _build_nc()
    res = run_bass_kernel_spmd(nc, in_maps, core_ids=list(range(NCORES)), trace=trace)
    # out_dev [pr, 128, NB, 2, 64] -> [pr, h, NB, 128, 64] -> [4, L, E]
    outs = np.stack(
        [
            np.asarray(res.results[c]["out"], dtype=np.float32)
            .transpose(0, 3, 2, 1, 4)
            .reshape(BH_PER_CORE, L, E)
            for c in range(NCORES)
        ]
    )
    full = outs.reshape(B, H, L, E)
    return full, res


def kernel(q, k, v, win_len):
    out, _ = _run(q, k, v, win_len, trace=False)
    return out
